# revision 1
# baseline (speedup 1.0000x reference)
"""Trainium2 Bass kernel for nn_DCTFeatureModel.

Math: the reference pipeline (3D DCT-II over [time-in-bin, H, W], mean over
DCT bins, full-receptive-field Conv3d, bias, LeakyReLU) is linear up to the
LeakyReLU, so everything folds into a single small matmul:

    feat[b,s,o] = LeakyReLU( sum_{c,t,i,j} x[b,s,c,t,i,j] * Weff[s,o,t,i,j]
                             + bias[s,o] )
    Weff[s,o,t,i,j] = (1/8) * sum_{f,p,q} Ct[f,t] Cs[p,i] Cs[q,j] W[s,o,f,p,q]

Weff is tiny and computed on host. The device kernel is memory-bound:
stream x (bf16, 8.4 MB/core at the ~352 GB/s HBM fair share), reduce over
the 8 DCT bins (c) with a pairwise tree of unit-stride bf16 DVE adds
(2x packed mode), then w-stationary matmuls into PSUM[o, b] and a single
fused bias+LeakyReLU (Prelu with per-partition alpha — the HW Lrelu
hardwires slope 0.01) on the scalar engine per subwindow.

Tail engineering: the contraction for s1 is split unevenly into 12+4
chunk groups. The big (3 MB) s1 block streams FIRST and the small (1 MB)
s1 block LAST, so after the final byte lands only a half-width tree
(3 short adds), a col-split final add, 4 matmuls and the Prelu remain.
All DMAs are contiguous column slices of one [128, 32768] bf16 tensor on
the sync HWDGE ring (strict FIFO = deterministic landing order); w/bias
lead the ring so the scalar engine's ACT_TABLE_LOAD stays off the
stream-start path.

Sharding: pure data-parallel over batch, 1024/8 = 128 rows per core.
"""

from contextlib import ExitStack

import ml_dtypes
import numpy as np

import concourse.bacc as bacc
import concourse.tile as tile
from concourse import mybir
from concourse.bass_utils import run_bass_kernel_spmd

# Problem shapes (hardcoded per contract)
B = 1024
NCORES = 8
BS = B // NCORES          # 128 batch rows per core
NSW = 2                   # subwindows
NBINS = 8                 # DCT bins (mean-reduced)
NDCT = 32                 # time points per bin
HW = 8
NF = 64                   # conv output filters per subwindow
K = NDCT * HW * HW        # 2048 contraction elements per (s, c)
P = 128                   # partitions
NCHUNK = K // P           # 16 k-chunks of 128 per subwindow
OUT_F = NSW * NF          # 128 output features
SLOPE = 0.02
TOTCOLS = NSW * NBINS * NCHUNK * P * BS // P  # 32768 bf16 cols per partition

# blocks in STREAM order: (s, chunk_lo, chunk_hi); the 3.25 MB s1 block
# first, the 0.75 MB s1 block last (short post-stream tail)
BLOCKS = [(1, 0, 13), (0, 0, 8), (0, 8, 16), (1, 13, 16)]

F32 = mybir.dt.float32
BF16 = mybir.dt.bfloat16
NP_BF16 = ml_dtypes.bfloat16

_cached = None
last_results = None


def _dct2(N):
    n = np.arange(N, dtype=np.float64)
    k = np.arange(N, dtype=np.float64)
    return 2.0 * np.cos(np.pi * (2.0 * n[None, :] + 1.0) * k[:, None] / (2.0 * N))


def _kernel_body(tc, x, w, bias, out):
    """x: [P, TOTCOLS] bf16, column-concatenated half-blocks in stream order;
    each block (s, lo, hi) is laid [kin, (c, chin in lo:hi, b)].
    w: [P, NSW*NCHUNK*NF] bf16 lhsT chunks. bias: [OUT_F, 1] f32 (s,o)-major.
    out: [OUT_F, BS] f32."""
    nc = tc.nc
    with ExitStack() as ctx:
        const_pool = ctx.enter_context(tc.tile_pool(name="const", bufs=1))
        xpool = ctx.enter_context(tc.tile_pool(name="xp", bufs=1))
        upool = ctx.enter_context(tc.tile_pool(name="up", bufs=1))
        zpool = ctx.enter_context(tc.tile_pool(name="zp", bufs=1))
        pft_pool = ctx.enter_context(tc.tile_pool(name="pft", bufs=1, space="PSUM"))

        # consts lead the sync ring: w is needed before the first matmul, and
        # issuing it here keeps the scalar engine's ACT_TABLE_LOAD (~1.3 us)
        # off the stream-start critical path
        w_sb = const_pool.tile([P, NSW * NCHUNK * NF], BF16)
        nc.sync.dma_start(out=w_sb, in_=w)
        bias_sb = const_pool.tile([OUT_F, 1], F32)
        nc.sync.dma_start(out=bias_sb, in_=bias)
        # Prelu slope as per-partition AP (HW Lrelu ignores alpha)
        alpha_sb = const_pool.tile([OUT_F, 1], F32)
        nc.gpsimd.memset(alpha_sb, SLOPE)

        out_sb = const_pool.tile([OUT_F, BS], F32)
        psum_feat = [
            pft_pool.tile([NF, BS], F32, tag=f"feat{s}", name=f"psum_feat{s}")
            for s in range(NSW)
        ]

        # stream the half-blocks (c0-3 | c4-7 of each block), strict FIFO
        halves = []
        off = 0
        for bi, (s, lo, hi) in enumerate(BLOCKS):
            gw = (hi - lo) * P            # z width for this block
            hw_cols = 4 * gw              # half-block width (4 c-slices)
            pair = []
            for h in range(2):
                t = xpool.tile([P, hw_cols], BF16, tag=f"x{bi}{h}", name=f"x{bi}{h}")
                nc.sync.dma_start(out=t, in_=x[:, off:off + hw_cols])
                off += hw_cols
                pair.append(t)
            halves.append(pair)

        for bi, (s, lo, hi) in enumerate(BLOCKS):
            gw = (hi - lo) * P
            h0, h1 = halves[bi]
            # pairwise c-tree, unit-stride bf16 adds (DVE 2x packed mode);
            # first-half partials overlap the second half's DMA
            u01 = upool.tile([P, gw], BF16, tag=f"u01_{bi}", name=f"u01_{bi}")
            # gpsimd is ~4x slower than DVE and its output feeds v0, so it
            # only helps on the first block whose DMA window has slack;
            # elsewhere it head-of-line-blocks the in-order DVE queue
            eng = nc.gpsimd if bi == 0 else nc.vector
            eng.tensor_add(out=u01, in0=h0[:, 0:gw], in1=h0[:, gw:2 * gw])
            u23 = upool.tile([P, gw], BF16, tag=f"u23_{bi}", name=f"u23_{bi}")
            nc.vector.tensor_add(out=u23, in0=h0[:, 2 * gw:3 * gw], in1=h0[:, 3 * gw:4 * gw])
            v0 = upool.tile([P, gw], BF16, tag=f"v0_{bi}", name=f"v0_{bi}")
            nc.vector.tensor_add(out=v0, in0=u01, in1=u23)
            u45 = upool.tile([P, gw], BF16, tag=f"u45_{bi}", name=f"u45_{bi}")
            nc.vector.tensor_add(out=u45, in0=h1[:, 0:gw], in1=h1[:, gw:2 * gw])
            u67 = upool.tile([P, gw], BF16, tag=f"u67_{bi}", name=f"u67_{bi}")
            nc.vector.tensor_add(out=u67, in0=h1[:, 2 * gw:3 * gw], in1=h1[:, 3 * gw:4 * gw])
            v1 = upool.tile([P, gw], BF16, tag=f"v1_{bi}", name=f"v1_{bi}")
            nc.vector.tensor_add(out=v1, in0=u45, in1=u67)
            z = zpool.tile([P, gw], BF16, tag=f"z_{bi}", name=f"z_{bi}")
            # final add col-split (chunk-aligned): the first part's matmuls
            # overlap the second part's add
            nch = gw // P
            cuts = [0, (nch + 1) // 2 * P, gw] if nch > 1 else [0, gw]
            for zi in range(len(cuts) - 1):
                a, b_ = cuts[zi], cuts[zi + 1]
                nc.vector.tensor_add(
                    out=z[:, a:b_], in0=v0[:, a:b_], in1=v1[:, a:b_]
                )
                # w-stationary matmuls: psum[o, b] += w_chunk.T @ z_chunk
                for chin in range(a // P, b_ // P):
                    ch = lo + chin
                    nc.tensor.matmul(
                        psum_feat[s],
                        lhsT=w_sb[:, (s * NCHUNK + ch) * NF:(s * NCHUNK + ch + 1) * NF],
                        rhs=z[:, chin * P:(chin + 1) * P],
                        start=(ch == 0),
                        stop=(ch == NCHUNK - 1),
                    )
            if hi == NCHUNK:
                # bias + LeakyReLU in one ACT instruction: Prelu(psum + bias)
                nc.scalar.activation(
                    out_sb[s * NF:(s + 1) * NF, :],
                    psum_feat[s],
                    mybir.ActivationFunctionType.Prelu,
                    bias=bias_sb[s * NF:(s + 1) * NF, :],
                    alpha=alpha_sb[s * NF:(s + 1) * NF, :],
                )
                nc.sync.dma_start(
                    out=out[s * NF:(s + 1) * NF, :],
                    in_=out_sb[s * NF:(s + 1) * NF, :],
                )


def _build():
    global _cached
    if _cached is not None:
        return _cached
    nc = bacc.Bacc(
        "TRN2",
        target_bir_lowering=False,
        debug=False,
        enable_asserts=False,
        num_devices=NCORES,
    )
    x_ap = nc.dram_tensor("x", [P, TOTCOLS], BF16, kind="ExternalInput").ap()
    w_ap = nc.dram_tensor("w", [P, NSW * NCHUNK * NF], BF16, kind="ExternalInput").ap()
    b_ap = nc.dram_tensor("bias", [OUT_F, 1], F32, kind="ExternalInput").ap()
    out_ap = nc.dram_tensor("out", [OUT_F, BS], F32, kind="ExternalOutput").ap()
    with tile.TileContext(nc, trace_sim=False) as tc:
        _kernel_body(tc, x_ap, w_ap, b_ap, out_ap)
    nc.compile()
    _cached = nc
    return nc


def kernel(x, W, b):
    global last_results
    assert x.shape == (B, 1, NSW * NBINS * NDCT, HW, HW), x.shape
    nc = _build()

    # Host-side folding of the DCT matrices into the conv weights (tiny).
    Ct = _dct2(NDCT)                       # [f, t]
    Cs = _dct2(HW)                         # [p, i]
    Weff = np.einsum(
        "ft,pi,qj,sofpq->sotij", Ct, Cs, Cs, W.astype(np.float64), optimize=True
    ) / float(NBINS)
    Weff_k = Weff.reshape(NSW, NF, K)      # [s, o, k]
    # lhsT chunk layout: w[p, (s*NCHUNK+ch)*NF + o] = Weff_k[s, o, ch*128 + p]
    w_dev = np.ascontiguousarray(
        Weff_k.reshape(NSW, NF, NCHUNK, P).transpose(3, 0, 2, 1).reshape(P, NSW * NCHUNK * NF)
    ).astype(NP_BF16)
    bias_dev = np.ascontiguousarray(b.reshape(OUT_F, 1)).astype(np.float32)

    x2 = x.reshape(B, NSW, NBINS, NCHUNK, P)  # (b, s, c, ch, kin)
    in_maps = []
    for i in range(NCORES):
        xs = x2[i * BS:(i + 1) * BS]
        cols = []
        for s, lo, hi in BLOCKS:
            t = xs[:, s, :, lo:hi, :].transpose(3, 1, 2, 0)  # [kin, c, chin, b]
            t = t.reshape(P, NBINS * (hi - lo) * BS)
            cols.append(t[:, :t.shape[1] // 2])
            cols.append(t[:, t.shape[1] // 2:])
        xt = np.ascontiguousarray(np.concatenate(cols, axis=1)).astype(NP_BF16)
        in_maps.append({"x": xt, "w": w_dev, "bias": bias_dev})
    res = run_bass_kernel_spmd(nc, in_maps, core_ids=list(range(NCORES)))
    last_results = res
    # device emits [s*64+o, b] per core; transpose back to [b, s*64+o]
    return np.concatenate([r["out"].T for r in res.results], axis=0)



# revision 5
# speedup vs baseline: 1.2470x; 1.2470x over previous
"""Trainium2 Bass kernel for nn_DCTFeatureModel.

Math: the reference pipeline (3D DCT-II over [time-in-bin, H, W], mean over
DCT bins, full-receptive-field Conv3d, bias, LeakyReLU) is linear up to the
LeakyReLU, so everything folds into a single small matmul:

    feat[b,s,o] = LeakyReLU( sum_{c,t,i,j} x[b,s,c,t,i,j] * Weff[s,o,t,i,j]
                             + bias[s,o] )
    Weff[s,o,t,i,j] = (1/8) * sum_{f,p,q} Ct[f,t] Cs[p,i] Cs[q,j] W[s,o,f,p,q]

Weff is tiny and computed on host.

Quantization (rel-err budget 2e-2; this lands ~5e-3): x is shipped at
1 byte/element in "count units" x/s (s = 4/127): DCT bins c0..c5 as int8,
c6/c7 as fp8e4. Error-feedback quantization along the c chain (fp8 slices
first, int8 after) makes the device's c-sum accurate to the final int8
residual (~0.3%). The dequant scale s is folded into the fp16 weights.

Device pipeline per core (measured rates in comments):
- stream 4.2 MB on the sync HWDGE ring in 16 pieces (4 per block), order
  [c45, c01, c23, c67] per block               (~12-13 us at HBM line rate)
- DVE: u01=c0+c1, u23=c2+c3 (int8->fp16, 1.08 ns/col), v0=u01+u23
  (fp16 2x, 0.55 ns/col)                                        (~11.5 us)
- GpSimd: u45=c4+c5 for blocks 0-2 (2.8 ns/col)                 (~10.4 us)
- PE: per chunk 4 matmuls x 128 cols into PSUM[o,b]: u45, c6, c7 (fp8 rhs,
  fp16 lhsT - HW-verified exact), v0. 0.83 ns/col + hidden LDW  (~13.6 us)
- ACT: Prelu table primed at start; s0's Prelu+out DMA fire mid-stream,
  s1's in the ~2 us post-stream tail.

Sharding: pure data-parallel over batch, 1024/8 = 128 rows per core.
"""

from contextlib import ExitStack

import ml_dtypes
import numpy as np

import concourse.bacc as bacc
import concourse.tile as tile
from concourse import mybir
from concourse.bass_utils import run_bass_kernel_spmd

# Problem shapes (hardcoded per contract)
B = 1024
NCORES = 8
BS = B // NCORES          # 128 batch rows per core
NSW = 2                   # subwindows
NBINS = 8                 # DCT bins (mean-reduced)
NDCT = 32                 # time points per bin
HW = 8
NF = 64                   # conv output filters per subwindow
K = NDCT * HW * HW        # 2048 contraction elements per (s, c)
P = 128                   # partitions
NCHUNK = K // P           # 16 k-chunks of 128 per subwindow
OUT_F = NSW * NF          # 128 output features
SLOPE = 0.02
QSCALE = 4.0 / 127.0      # int8 quant scale (4-sigma clip)

# blocks in STREAM order: (s, chunk_lo, chunk_hi); big s1 block first,
# 3-chunk s1 block last (short post-stream tail). s0 finishes mid-stream.
BLOCKS = [(1, 0, 13), (0, 0, 8), (0, 8, 16), (1, 13, 16)]

INT_COLS = sum((hi - lo) * P * 6 * BS // P for _, lo, hi in BLOCKS)  # 3*2*gw each
FP8_COLS = sum((hi - lo) * P * 2 * BS // P for _, lo, hi in BLOCKS)

F32 = mybir.dt.float32
F16 = mybir.dt.float16
I8 = mybir.dt.int8
FP8 = mybir.dt.float8e4
NP_F16 = np.float16
NP_FP8 = ml_dtypes.float8_e4m3fn

_cached = None
last_results = None


def _dct2(N):
    n = np.arange(N, dtype=np.float64)
    k = np.arange(N, dtype=np.float64)
    return 2.0 * np.cos(np.pi * (2.0 * n[None, :] + 1.0) * k[:, None] / (2.0 * N))


def _kernel_body(tc, xi, xf, w, bias, out):
    """xi: [P, INT_COLS] int8 - per block, pieces (c45, c01, c23), each laid
    [kin, (c, chin, b)]. xf: [P, FP8_COLS] fp8e4 - per block piece (c67).
    w: [P, NSW*NCHUNK*NF] fp16 lhsT chunks (x-scale folded in).
    bias: [OUT_F, 1] f32 (s,o)-major. out: [OUT_F, BS] f32."""
    nc = tc.nc
    with ExitStack() as ctx:
        const_pool = ctx.enter_context(tc.tile_pool(name="const", bufs=1))
        xpool = ctx.enter_context(tc.tile_pool(name="xp", bufs=1))
        upool = ctx.enter_context(tc.tile_pool(name="up", bufs=1))
        pft_pool = ctx.enter_context(tc.tile_pool(name="pft", bufs=1, space="PSUM"))

        # consts ride the scalar (ACT) HWDGE ring - off the x stream ring
        w_sb = const_pool.tile([P, NSW * NCHUNK * NF], F16)
        nc.scalar.dma_start(out=w_sb, in_=w)
        bias_sb = const_pool.tile([OUT_F, 1], F32)
        nc.scalar.dma_start(out=bias_sb, in_=bias)
        # Prelu slope as per-partition AP (HW Lrelu hardwires slope 0.01)
        alpha_sb = const_pool.tile([OUT_F, 1], F32)
        nc.gpsimd.memset(alpha_sb, SLOPE)
        # prime the ACT Prelu table now so ACT_TABLE_LOAD (~1.3us) runs
        # during the stream, not in the tail
        prime_sb = const_pool.tile([OUT_F, 1], F32)
        nc.scalar.activation(
            prime_sb,
            alpha_sb,
            mybir.ActivationFunctionType.Prelu,
            bias=alpha_sb,
            alpha=alpha_sb,
        )

        out_sb = const_pool.tile([OUT_F, BS], F32)
        psum_feat = [
            pft_pool.tile([NF, BS], F32, tag=f"feat{s}", name=f"psum_feat{s}")
            for s in range(NSW)
        ]

        # stream pieces, strict FIFO on the sync ring:
        # per block [c45, c01, c23 (int8), c67 (fp8)]
        pieces = []
        ioff = foff = 0
        for bi, (s, lo, hi) in enumerate(BLOCKS):
            gw = (hi - lo) * P
            row = {}
            for nm in ("c45", "c01", "c23"):
                t = xpool.tile([P, 2 * gw], I8, tag=f"x{bi}{nm}", name=f"x{bi}{nm}")
                nc.sync.dma_start(out=t, in_=xi[:, ioff:ioff + 2 * gw])
                ioff += 2 * gw
                row[nm] = t
            t = xpool.tile([P, 2 * gw], FP8, tag=f"x{bi}c67", name=f"x{bi}c67")
            nc.sync.dma_start(out=t, in_=xf[:, foff:foff + 2 * gw])
            foff += 2 * gw
            row["c67"] = t
            pieces.append(row)

        # MM bookkeeping for start/stop flags per psum accumulation group
        n_mm = {0: 0, 1: 0}
        for s, lo, hi in BLOCKS:
            n_mm[s] += 4 * (hi - lo)
        mm_seen = {0: 0, 1: 0}

        def mm(s, lhsT, rhs):
            mm_seen[s] += 1
            nc.tensor.matmul(
                psum_feat[s],
                lhsT=lhsT,
                rhs=rhs,
                start=(mm_seen[s] == 1),
                stop=(mm_seen[s] == n_mm[s]),
            )

        def finish(s):
            nc.scalar.activation(
                out_sb[s * NF:(s + 1) * NF, :],
                psum_feat[s],
                mybir.ActivationFunctionType.Prelu,
                bias=bias_sb[s * NF:(s + 1) * NF, :],
                alpha=alpha_sb[s * NF:(s + 1) * NF, :],
            )
            nc.sync.dma_start(
                out=out[s * NF:(s + 1) * NF, :],
                in_=out_sb[s * NF:(s + 1) * NF, :],
            )

        for bi, (s, lo, hi) in enumerate(BLOCKS):
            gw = (hi - lo) * P
            nch = hi - lo
            pc = pieces[bi]
            last = bi == len(BLOCKS) - 1
            # u45 on gpsimd for blocks 0-2 (arrives first, Q7 is slow);
            # on DVE for the last block so the tail has no gpsimd dependency
            u45 = upool.tile([P, gw], F16, tag=f"u45_{bi}", name=f"u45_{bi}")
            eng = nc.vector if last else nc.gpsimd
            eng.tensor_add(out=u45, in0=pc["c45"][:, 0:gw], in1=pc["c45"][:, gw:2 * gw])
            u01 = upool.tile([P, gw], F16, tag=f"u01_{bi}", name=f"u01_{bi}")
            nc.vector.tensor_add(out=u01, in0=pc["c01"][:, 0:gw], in1=pc["c01"][:, gw:2 * gw])
            u23 = upool.tile([P, gw], F16, tag=f"u23_{bi}", name=f"u23_{bi}")
            nc.vector.tensor_add(out=u23, in0=pc["c23"][:, 0:gw], in1=pc["c23"][:, gw:2 * gw])
            v0 = upool.tile([P, gw], F16, tag=f"v0_{bi}", name=f"v0_{bi}")
            nc.vector.tensor_add(out=v0, in0=u01, in1=u23)

            def w_ch(ch):
                return w_sb[:, (s * NCHUNK + ch) * NF:(s * NCHUNK + ch + 1) * NF]

            # PE issue order = expected operand readiness (in-order queue):
            # blocks 0-2: c67 piece lands mid-block, v0 soon after, gpsimd
            # u45 last. Last block: u45/v0 (DVE) early, c67 piece last.
            if not last:
                groups = [("c6", None), ("c7", None), ("v0", v0), ("u45", u45)]
            else:
                groups = [("u45", u45), ("v0", v0), ("c6", None), ("c7", None)]
            for nm, src in groups:
                for chin in range(nch):
                    ch = lo + chin
                    sl = slice(chin * P, (chin + 1) * P)
                    if nm == "c6":
                        rhs = pc["c67"][:, chin * P:(chin + 1) * P]
                    elif nm == "c7":
                        rhs = pc["c67"][:, gw + chin * P:gw + (chin + 1) * P]
                    else:
                        rhs = src[:, sl]
                    mm(s, w_ch(ch), rhs)
            if mm_seen[s] == n_mm[s]:
                finish(s)


def _build():
    global _cached
    if _cached is not None:
        return _cached
    nc = bacc.Bacc(
        "TRN2",
        target_bir_lowering=False,
        debug=False,
        enable_asserts=False,
        num_devices=NCORES,
    )
    xi_ap = nc.dram_tensor("xi", [P, INT_COLS], I8, kind="ExternalInput").ap()
    xf_ap = nc.dram_tensor("xf", [P, FP8_COLS], FP8, kind="ExternalInput").ap()
    w_ap = nc.dram_tensor("w", [P, NSW * NCHUNK * NF], F16, kind="ExternalInput").ap()
    b_ap = nc.dram_tensor("bias", [OUT_F, 1], F32, kind="ExternalInput").ap()
    out_ap = nc.dram_tensor("out", [OUT_F, BS], F32, kind="ExternalOutput").ap()
    with tile.TileContext(nc, trace_sim=False) as tc:
        _kernel_body(tc, xi_ap, xf_ap, w_ap, b_ap, out_ap)
    nc.compile()
    _cached = nc
    return nc


def kernel(x, W, b):
    global last_results
    assert x.shape == (B, 1, NSW * NBINS * NDCT, HW, HW), x.shape
    nc = _build()

    # Host-side folding of the DCT matrices into the conv weights (tiny).
    Ct = _dct2(NDCT)                       # [f, t]
    Cs = _dct2(HW)                         # [p, i]
    Weff = np.einsum(
        "ft,pi,qj,sofpq->sotij", Ct, Cs, Cs, W.astype(np.float64), optimize=True
    ) / float(NBINS)
    Weff_k = Weff.reshape(NSW, NF, K) * QSCALE   # fold dequant scale
    # lhsT chunk layout: w[p, (s*NCHUNK+ch)*NF + o] = Weff_k[s, o, ch*128 + p]
    w_dev = np.ascontiguousarray(
        Weff_k.reshape(NSW, NF, NCHUNK, P).transpose(3, 0, 2, 1).reshape(P, NSW * NCHUNK * NF)
    ).astype(NP_F16)
    bias_dev = np.ascontiguousarray(b.reshape(OUT_F, 1)).astype(np.float32)

    # Error-feedback quantization along c (fp8 slices first so their larger
    # residuals are absorbed by the later int8 slices).
    xs = x.reshape(B, NSW, NBINS, K).astype(np.float32) / np.float32(QSCALE)
    qi = np.zeros((B, NSW, 6, K), dtype=np.int8)
    qf = np.zeros((B, NSW, 2, K), dtype=NP_FP8)
    e = np.zeros((B, NSW, K), dtype=np.float32)
    for j, c in enumerate((6, 7)):
        v = xs[:, :, c] + e
        qc = v.astype(NP_FP8)
        e = v - qc.astype(np.float32)
        qf[:, :, j] = qc
    for c in range(6):
        v = xs[:, :, c] + e
        qc = np.clip(np.round(v), -127, 127)
        e = v - qc
        qi[:, :, c] = qc.astype(np.int8)

    # reshape to (b, s, c, ch, kin) views
    qi = qi.reshape(B, NSW, 6, NCHUNK, P)
    qf = qf.reshape(B, NSW, 2, NCHUNK, P)

    in_maps = []
    for i in range(NCORES):
        sl = slice(i * BS, (i + 1) * BS)
        icols, fcols = [], []
        for s, lo, hi in BLOCKS:
            for c0, c1 in ((4, 5), (0, 1), (2, 3)):
                t = qi[sl, s, (c0, c1), lo:hi, :]        # [b, 2, ch, kin]
                icols.append(t.transpose(3, 1, 2, 0).reshape(P, 2 * (hi - lo) * BS))
            t = qf[sl, s, :, lo:hi, :]                   # [b, 2, ch, kin]
            fcols.append(t.transpose(3, 1, 2, 0).reshape(P, 2 * (hi - lo) * BS))
        xi_dev = np.ascontiguousarray(np.concatenate(icols, axis=1))
        xf_dev = np.ascontiguousarray(np.concatenate(fcols, axis=1))
        in_maps.append({"xi": xi_dev, "xf": xf_dev, "w": w_dev, "bias": bias_dev})
    res = run_bass_kernel_spmd(nc, in_maps, core_ids=list(range(NCORES)))
    last_results = res
    # device emits [s*64+o, b] per core; transpose back to [b, s*64+o]
    return np.concatenate([r["out"].T for r in res.results], axis=0)


# revision 7
# speedup vs baseline: 1.3158x; 1.0552x over previous
"""Trainium2 Bass kernel for nn_DCTFeatureModel.

Math: the reference pipeline (3D DCT-II over [time-in-bin, H, W], mean over
DCT bins, full-receptive-field Conv3d, bias, LeakyReLU) is linear up to the
LeakyReLU, so everything folds into a single small matmul:

    feat[b,s,o] = LeakyReLU( sum_{c,t,i,j} x[b,s,c,t,i,j] * Weff[s,o,t,i,j]
                             + bias[s,o] )
    Weff[s,o,t,i,j] = (1/8) * sum_{f,p,q} Ct[f,t] Cs[p,i] Cs[q,j] W[s,o,f,p,q]

Weff is tiny and computed on host.

Quantization (rel-err budget 2e-2; this lands ~3.3e-3): x ships at 1
byte/element in "count units" x/s (s = 4/127): DCT bins c0..c3 as int8,
c4..c7 as fp8e4. Error-feedback quantization along the c chain (fp8 slices
first, int8 after) makes the device's c-sum accurate to the final int8
residual (~0.3%). The dequant scale s is folded into the fp16 weights.

Device pipeline per core (rates measured on this HW):
- stream 4.2 MB on the sync HWDGE ring, 12 pieces: per block an int8 c01
  piece, an int8 c23 piece, and a merged fp8 c4..c7 piece
- the PE contracts fp8 slices RAW (fp16 lhsT x fp8 rhs matmuls accumulate
  the c-sum in PSUM; HW-verified exact) - 128 of the ~180 matmuls, LDW
  fully hidden, 56-107 ns per 128-col matmul
- DVE does u01=c0+c1 everywhere (int8->fp16, ~1.3 ns/col), u23 for the two
  late blocks, and v0=u01+u23 for the two early blocks (fp16 2x)
- GpSimd adds u23 for the two early blocks (~2.5 ns/col)
- ACT: Prelu table primed at start; s0 finishes mid-stream (out DMA on the
  scalar ring), s1 in the ~2.5 us post-stream tail (out on the sync ring)

Sharding: pure data-parallel over batch, 1024/8 = 128 rows per core.
"""

from contextlib import ExitStack

import ml_dtypes
import numpy as np

import concourse.bacc as bacc
import concourse.tile as tile
from concourse import mybir
from concourse.bass_utils import run_bass_kernel_spmd

# Problem shapes (hardcoded per contract)
B = 1024
NCORES = 8
BS = B // NCORES          # 128 batch rows per core
NSW = 2                   # subwindows
NBINS = 8                 # DCT bins (mean-reduced)
NDCT = 32                 # time points per bin
HW = 8
NF = 64                   # conv output filters per subwindow
K = NDCT * HW * HW        # 2048 contraction elements per (s, c)
P = 128                   # partitions
NCHUNK = K // P           # 16 k-chunks of 128 per subwindow
OUT_F = NSW * NF          # 128 output features
SLOPE = 0.02
QSCALE = 4.0 / 127.0      # int8 quant scale (4-sigma clip)

# blocks in STREAM order: (s, chunk_lo, chunk_hi); big s1 block first,
# 3-chunk s1 block last (short post-stream tail). s0 finishes mid-stream.
BLOCKS = [(1, 0, 13), (0, 0, 8), (0, 8, 16), (1, 13, 16)]
GWS = [(hi - lo) * P for _, lo, hi in BLOCKS]

INT_COLS = sum(4 * gw for gw in GWS)   # c0..c3, two 2*gw pieces per block
FP8_COLS = sum(4 * gw for gw in GWS)   # c4..c7, one 4*gw piece per block

F32 = mybir.dt.float32
F16 = mybir.dt.float16
I8 = mybir.dt.int8
FP8 = mybir.dt.float8e4
NP_F16 = np.float16
NP_FP8 = ml_dtypes.float8_e4m3fn

# which blocks reduce u01+u23 -> v0 on DVE (early blocks; late blocks send
# u01/u23 straight to the PE so the DVE chain isn't in the tail)
HAS_V0 = [True, True, False, False]
# which blocks compute u23 on gpsimd (early blocks only - Q7 is ~2x slower)
U23_GPSIMD = [True, True, False, False]

_cached = None
last_results = None


def _dct2(N):
    n = np.arange(N, dtype=np.float64)
    k = np.arange(N, dtype=np.float64)
    return 2.0 * np.cos(np.pi * (2.0 * n[None, :] + 1.0) * k[:, None] / (2.0 * N))


def _kernel_body(tc, xi, xf, w, bias, out):
    """xi: [P, INT_COLS] int8 - per block, pieces (c01, c23), each laid
    [kin, (c, chin, b)]. xf: [P, FP8_COLS] fp8e4 - per block piece (c4..c7).
    w: [P, NSW*NCHUNK*NF] fp16 lhsT chunks (x-scale folded in).
    bias: [OUT_F, 1] f32 (s,o)-major. out: [OUT_F, BS] f32."""
    nc = tc.nc
    with ExitStack() as ctx:
        const_pool = ctx.enter_context(tc.tile_pool(name="const", bufs=1))
        xpool = ctx.enter_context(tc.tile_pool(name="xp", bufs=1))
        upool = ctx.enter_context(tc.tile_pool(name="up", bufs=1))
        pft_pool = ctx.enter_context(tc.tile_pool(name="pft", bufs=1, space="PSUM"))

        # consts ride the scalar (ACT) HWDGE ring - off the x stream ring
        w_sb = const_pool.tile([P, NSW * NCHUNK * NF], F16)
        nc.scalar.dma_start(out=w_sb, in_=w)
        bias_sb = const_pool.tile([OUT_F, 1], F32)
        nc.scalar.dma_start(out=bias_sb, in_=bias)
        # Prelu slope as per-partition AP (HW Lrelu hardwires slope 0.01)
        alpha_sb = const_pool.tile([OUT_F, 1], F32)
        nc.gpsimd.memset(alpha_sb, SLOPE)
        # prime the ACT Prelu table so ACT_TABLE_LOAD (~1.3us) runs early
        prime_sb = const_pool.tile([OUT_F, 1], F32)
        nc.scalar.activation(
            prime_sb,
            alpha_sb,
            mybir.ActivationFunctionType.Prelu,
            bias=alpha_sb,
            alpha=alpha_sb,
        )

        out_sb = const_pool.tile([OUT_F, BS], F32)
        psum_feat = [
            pft_pool.tile([NF, BS], F32, tag=f"feat{s}", name=f"psum_feat{s}")
            for s in range(NSW)
        ]

        # stream pieces, strict FIFO on the sync ring:
        # per block [c01, c23 (int8), c4567 (fp8)]
        pieces = []
        ioff = foff = 0
        for bi, (s, lo, hi) in enumerate(BLOCKS):
            gw = GWS[bi]
            row = {}
            for nm in ("c01", "c23"):
                t = xpool.tile([P, 2 * gw], I8, tag=f"x{bi}{nm}", name=f"x{bi}{nm}")
                nc.sync.dma_start(out=t, in_=xi[:, ioff:ioff + 2 * gw])
                ioff += 2 * gw
                row[nm] = t
            t = xpool.tile([P, 4 * gw], FP8, tag=f"x{bi}f8", name=f"x{bi}f8")
            nc.sync.dma_start(out=t, in_=xf[:, foff:foff + 4 * gw])
            foff += 4 * gw
            row["f8"] = t
            pieces.append(row)

        # --- DVE / gpsimd add tree -----------------------------------
        def pair_add(eng, nm, bi, piece):
            gw = GWS[bi]
            t = upool.tile([P, gw], F16, tag=f"{nm}_{bi}", name=f"{nm}_{bi}")
            eng.tensor_add(out=t, in0=piece[:, 0:gw], in1=piece[:, gw:2 * gw])
            return t

        u01 = [None] * 4
        u23 = [None] * 4
        v0 = [None] * 4

        # gpsimd queue (in-order): u23 for early blocks
        for bi in range(4):
            if U23_GPSIMD[bi]:
                u23[bi] = pair_add(nc.gpsimd, "u23", bi, pieces[bi]["c23"])
        # DVE queue in expected-readiness order
        u01[0] = pair_add(nc.vector, "u01", 0, pieces[0]["c01"])
        u01[1] = pair_add(nc.vector, "u01", 1, pieces[1]["c01"])
        v0[0] = upool.tile([P, GWS[0]], F16, tag="v0_0", name="v0_0")
        nc.vector.tensor_add(out=v0[0], in0=u01[0], in1=u23[0])
        u01[2] = pair_add(nc.vector, "u01", 2, pieces[2]["c01"])
        u23[2] = pair_add(nc.vector, "u23", 2, pieces[2]["c23"])
        u01[3] = pair_add(nc.vector, "u01", 3, pieces[3]["c01"])
        u23[3] = pair_add(nc.vector, "u23", 3, pieces[3]["c23"])
        v0[1] = upool.tile([P, GWS[1]], F16, tag="v0_1", name="v0_1")
        nc.vector.tensor_add(out=v0[1], in0=u01[1], in1=u23[1])

        # --- matmuls --------------------------------------------------
        # per-psum MM counts for start/stop flags
        n_mm = {0: 0, 1: 0}
        for bi, (s, lo, hi) in enumerate(BLOCKS):
            nch = hi - lo
            n_mm[s] += 4 * nch + (nch if HAS_V0[bi] else 2 * nch)
        mm_seen = {0: 0, 1: 0}

        def mm(s, ch, rhs):
            mm_seen[s] += 1
            nc.tensor.matmul(
                psum_feat[s],
                lhsT=w_sb[:, (s * NCHUNK + ch) * NF:(s * NCHUNK + ch + 1) * NF],
                rhs=rhs,
                start=(mm_seen[s] == 1),
                stop=(mm_seen[s] == n_mm[s]),
            )

        def fp8_group(bi):
            s, lo, hi = BLOCKS[bi]
            gw = GWS[bi]
            f8 = pieces[bi]["f8"]
            for j in range(4):
                for chin in range(hi - lo):
                    mm(s, lo + chin, f8[:, j * gw + chin * P:j * gw + (chin + 1) * P])

        def red_group(bi, src):
            s, lo, hi = BLOCKS[bi]
            for chin in range(hi - lo):
                mm(s, lo + chin, src[:, chin * P:(chin + 1) * P])

        # PE queue in expected-readiness order
        fp8_group(0)                       # b0 fp8 lands ~1/3 into stream
        fp8_group(1)
        red_group(0, v0[0])                # v0(b0) after gpsimd u23(b0)
        fp8_group(2)
        red_group(2, u01[2])
        red_group(2, u23[2])
        fp8_group(3)
        red_group(1, v0[1])                # late (gpsimd u23(b1)); s0 stop here
        red_group(3, u01[3])
        red_group(3, u23[3])               # s1 stop here

        assert mm_seen[0] == n_mm[0] and mm_seen[1] == n_mm[1]

        # --- epilogue: Prelus back-to-back on ACT, outs on sync -------
        for s in range(NSW):
            nc.scalar.activation(
                out_sb[s * NF:(s + 1) * NF, :],
                psum_feat[s],
                mybir.ActivationFunctionType.Prelu,
                bias=bias_sb[s * NF:(s + 1) * NF, :],
                alpha=alpha_sb[s * NF:(s + 1) * NF, :],
            )
            nc.sync.dma_start(
                out=out[s * NF:(s + 1) * NF, :],
                in_=out_sb[s * NF:(s + 1) * NF, :],
            )


def _build():
    global _cached
    if _cached is not None:
        return _cached
    nc = bacc.Bacc(
        "TRN2",
        target_bir_lowering=False,
        debug=False,
        enable_asserts=False,
        num_devices=NCORES,
    )
    xi_ap = nc.dram_tensor("xi", [P, INT_COLS], I8, kind="ExternalInput").ap()
    xf_ap = nc.dram_tensor("xf", [P, FP8_COLS], FP8, kind="ExternalInput").ap()
    w_ap = nc.dram_tensor("w", [P, NSW * NCHUNK * NF], F16, kind="ExternalInput").ap()
    b_ap = nc.dram_tensor("bias", [OUT_F, 1], F32, kind="ExternalInput").ap()
    out_ap = nc.dram_tensor("out", [OUT_F, BS], F32, kind="ExternalOutput").ap()
    with tile.TileContext(nc, trace_sim=False) as tc:
        _kernel_body(tc, xi_ap, xf_ap, w_ap, b_ap, out_ap)
    nc.compile()
    _cached = nc
    return nc


def kernel(x, W, b):
    global last_results
    assert x.shape == (B, 1, NSW * NBINS * NDCT, HW, HW), x.shape
    nc = _build()

    # Host-side folding of the DCT matrices into the conv weights (tiny).
    Ct = _dct2(NDCT)                       # [f, t]
    Cs = _dct2(HW)                         # [p, i]
    Weff = np.einsum(
        "ft,pi,qj,sofpq->sotij", Ct, Cs, Cs, W.astype(np.float64), optimize=True
    ) / float(NBINS)
    Weff_k = Weff.reshape(NSW, NF, K) * QSCALE   # fold dequant scale
    # lhsT chunk layout: w[p, (s*NCHUNK+ch)*NF + o] = Weff_k[s, o, ch*128 + p]
    w_dev = np.ascontiguousarray(
        Weff_k.reshape(NSW, NF, NCHUNK, P).transpose(3, 0, 2, 1).reshape(P, NSW * NCHUNK * NF)
    ).astype(NP_F16)
    bias_dev = np.ascontiguousarray(b.reshape(OUT_F, 1)).astype(np.float32)

    # Error-feedback quantization along c (fp8 slices first so their larger
    # residuals are absorbed by the later int8 slices).
    xs = x.reshape(B, NSW, NBINS, K).astype(np.float32) / np.float32(QSCALE)
    qi = np.zeros((B, NSW, 4, K), dtype=np.int8)
    qf = np.zeros((B, NSW, 4, K), dtype=NP_FP8)
    e = np.zeros((B, NSW, K), dtype=np.float32)
    for j, c in enumerate((4, 5, 6, 7)):
        v = xs[:, :, c] + e
        qc = v.astype(NP_FP8)
        e = v - qc.astype(np.float32)
        qf[:, :, j] = qc
    for c in range(4):
        v = xs[:, :, c] + e
        qc = np.clip(np.round(v), -127, 127)
        e = v - qc
        qi[:, :, c] = qc.astype(np.int8)

    qi = qi.reshape(B, NSW, 4, NCHUNK, P)
    qf = qf.reshape(B, NSW, 4, NCHUNK, P)

    in_maps = []
    for i in range(NCORES):
        sl = slice(i * BS, (i + 1) * BS)
        icols, fcols = [], []
        for s, lo, hi in BLOCKS:
            for c0, c1 in ((0, 1), (2, 3)):
                t = qi[sl, s, (c0, c1), lo:hi, :]        # [b, 2, ch, kin]
                icols.append(t.transpose(3, 1, 2, 0).reshape(P, 2 * (hi - lo) * BS))
            t = qf[sl, s, :, lo:hi, :]                   # [b, 4, ch, kin]
            fcols.append(t.transpose(3, 1, 2, 0).reshape(P, 4 * (hi - lo) * BS))
        xi_dev = np.ascontiguousarray(np.concatenate(icols, axis=1))
        xf_dev = np.ascontiguousarray(np.concatenate(fcols, axis=1))
        in_maps.append({"xi": xi_dev, "xf": xf_dev, "w": w_dev, "bias": bias_dev})
    res = run_bass_kernel_spmd(nc, in_maps, core_ids=list(range(NCORES)))
    last_results = res
    # device emits [s*64+o, b] per core; transpose back to [b, s*64+o]
    return np.concatenate([r["out"].T for r in res.results], axis=0)


# revision 8
# speedup vs baseline: 1.3824x; 1.0506x over previous
"""Trainium2 Bass kernel for nn_DCTFeatureModel.

Math: the reference pipeline (3D DCT-II over [time-in-bin, H, W], mean over
DCT bins, full-receptive-field Conv3d, bias, LeakyReLU) is linear up to the
LeakyReLU, so everything folds into a single small matmul:

    feat[b,s,o] = LeakyReLU( sum_{c,t,i,j} x[b,s,c,t,i,j] * Weff[s,o,t,i,j]
                             + bias[s,o] )
    Weff[s,o,t,i,j] = (1/8) * sum_{f,p,q} Ct[f,t] Cs[p,i] Cs[q,j] W[s,o,f,p,q]

Weff is tiny and computed on host.

Quantization (rel-err budget 2e-2; this lands ~3.3e-3): x ships at 1
byte/element in "count units" x/s (s = 4/127): DCT bins c0..c3 as int8,
c4..c7 as fp8e4. Error-feedback quantization along the c chain (fp8 slices
first, int8 after) makes the device's c-sum accurate to the final int8
residual (~0.3%). The dequant scale s is folded into the fp16 weights.

Device schedule (v4, rates measured on this HW):
- 13 pieces on the sync HWDGE ring; block sizes s1:13+3, s0:11+5 so the
  late-arriving pieces carry little PE work; big fp8 pieces split in two
  for smooth PE feed. Piece-completion sems lag data by 1-3 us (slowest
  SDMA engine), so every consumer queue is issued in measured-readiness
  order.
- PE contracts the fp8 slices RAW (fp16 lhsT x fp8 rhs matmuls accumulate
  the c-sum in PSUM; HW-verified exact): ~170 matmuls, 56-107 ns each,
  LDWEIGHTS fully hidden.
- DVE: u01 everywhere (int8->fp16 ~1.3 ns/col), u23 late blocks, v0 early
  blocks (fp16 2x). GpSimd: u23 for the two big blocks (~2.5 ns/col).
- ACT Prelu table primed at start; both Prelus at the end back-to-back;
  outs on the sync ring.

Sharding: pure data-parallel over batch, 1024/8 = 128 rows per core.
"""

from contextlib import ExitStack

import ml_dtypes
import numpy as np

import concourse.bacc as bacc
import concourse.tile as tile
from concourse import mybir
from concourse.bass_utils import run_bass_kernel_spmd

# Problem shapes (hardcoded per contract)
B = 1024
NCORES = 8
BS = B // NCORES          # 128 batch rows per core
NSW = 2                   # subwindows
NBINS = 8                 # DCT bins (mean-reduced)
NDCT = 32                 # time points per bin
HW = 8
NF = 64                   # conv output filters per subwindow
K = NDCT * HW * HW        # 2048 contraction elements per (s, c)
P = 128                   # partitions
NCHUNK = K // P           # 16 k-chunks of 128 per subwindow
OUT_F = NSW * NF          # 128 output features
SLOPE = 0.02
QSCALE = 4.0 / 127.0      # int8 quant scale (4-sigma clip)

# blocks in STREAM order: (s, chunk_lo, chunk_hi)
BLOCKS = [(1, 0, 13), (0, 0, 11), (0, 11, 16), (1, 13, 16)]
GWS = [(hi - lo) * P for _, lo, hi in BLOCKS]

INT_COLS = sum(4 * gw for gw in GWS)   # c0..c3
FP8_COLS = sum(4 * gw for gw in GWS)   # c4..c7

F32 = mybir.dt.float32
F16 = mybir.dt.float16
I8 = mybir.dt.int8
FP8 = mybir.dt.float8e4
NP_F16 = np.float16
NP_FP8 = ml_dtypes.float8_e4m3fn

_cached = None
last_results = None


def _dct2(N):
    n = np.arange(N, dtype=np.float64)
    k = np.arange(N, dtype=np.float64)
    return 2.0 * np.cos(np.pi * (2.0 * n[None, :] + 1.0) * k[:, None] / (2.0 * N))


# host-side piece list: (kind, block, name, c-slices)
# int pieces come from xi (c-slices of 0..3), fp8 pieces from xf (0..3 = c4..c7)
PIECES = [
    ("i", 0, "c01", (0, 1)),
    ("i", 0, "c23", (2, 3)),
    ("f", 0, "f8a", (0, 1)),
    ("f", 0, "f8b", (2, 3)),
    ("i", 1, "c01", (0, 1)),
    ("i", 1, "c23", (2, 3)),
    ("f", 1, "f8a", (0, 1)),
    ("f", 1, "f8b", (2, 3)),
    ("i", 2, "int", (0, 1, 2, 3)),
    ("f", 2, "f8", (0, 1, 2, 3)),
    ("i", 3, "c01", (0, 1)),
    ("i", 3, "c23", (2, 3)),
    ("f", 3, "f8", (0, 1, 2, 3)),
]


def _kernel_body(tc, xi, xf, w, bias, out):
    nc = tc.nc
    with ExitStack() as ctx:
        const_pool = ctx.enter_context(tc.tile_pool(name="const", bufs=1))
        xpool = ctx.enter_context(tc.tile_pool(name="xp", bufs=1))
        upool = ctx.enter_context(tc.tile_pool(name="up", bufs=1))
        pft_pool = ctx.enter_context(tc.tile_pool(name="pft", bufs=1, space="PSUM"))

        # consts ride the scalar (ACT) HWDGE ring - off the x stream ring
        w_sb = const_pool.tile([P, NSW * NCHUNK * NF], F16)
        nc.scalar.dma_start(out=w_sb, in_=w)
        bias_sb = const_pool.tile([OUT_F, 1], F32)
        nc.scalar.dma_start(out=bias_sb, in_=bias)
        alpha_sb = const_pool.tile([OUT_F, 1], F32)
        nc.gpsimd.memset(alpha_sb, SLOPE)
        # prime the ACT Prelu table so ACT_TABLE_LOAD (~1.3us) runs early
        prime_sb = const_pool.tile([OUT_F, 1], F32)
        nc.scalar.activation(
            prime_sb,
            alpha_sb,
            mybir.ActivationFunctionType.Prelu,
            bias=alpha_sb,
            alpha=alpha_sb,
        )

        out_sb = const_pool.tile([OUT_F, BS], F32)
        psum_feat = [
            pft_pool.tile([NF, BS], F32, tag=f"feat{s}", name=f"psum_feat{s}")
            for s in range(NSW)
        ]

        # stream pieces, strict FIFO on the sync ring
        tiles = {}
        ioff = foff = 0
        for kind, bi, nm, cs in PIECES:
            gw = GWS[bi]
            ncols = len(cs) * gw
            if kind == "i":
                t = xpool.tile([P, ncols], I8, tag=f"x{bi}{nm}", name=f"x{bi}{nm}")
                nc.sync.dma_start(out=t, in_=xi[:, ioff:ioff + ncols])
                ioff += ncols
            else:
                t = xpool.tile([P, ncols], FP8, tag=f"x{bi}{nm}", name=f"x{bi}{nm}")
                nc.sync.dma_start(out=t, in_=xf[:, foff:foff + ncols])
                foff += ncols
            tiles[(bi, nm)] = t

        # --- adds (issue order = measured readiness; queues are in-order) --
        def pair_add(eng, nm, bi, src, off):
            gw = GWS[bi]
            t = upool.tile([P, gw], F16, tag=f"{nm}_{bi}", name=f"{nm}_{bi}")
            eng.tensor_add(
                out=t, in0=src[:, off * gw:(off + 1) * gw],
                in1=src[:, (off + 1) * gw:(off + 2) * gw],
            )
            return t

        # gpsimd: u23 for the two big blocks
        u23_0 = pair_add(nc.gpsimd, "u23", 0, tiles[(0, "c23")], 0)
        u23_1 = pair_add(nc.gpsimd, "u23", 1, tiles[(1, "c23")], 0)
        # DVE
        u01_0 = pair_add(nc.vector, "u01", 0, tiles[(0, "c01")], 0)
        v0_0 = upool.tile([P, GWS[0]], F16, tag="v0_0", name="v0_0")
        nc.vector.tensor_add(out=v0_0, in0=u01_0, in1=u23_0)
        u01_1 = pair_add(nc.vector, "u01", 1, tiles[(1, "c01")], 0)
        v0_1 = upool.tile([P, GWS[1]], F16, tag="v0_1", name="v0_1")
        nc.vector.tensor_add(out=v0_1, in0=u01_1, in1=u23_1)
        u01_2 = pair_add(nc.vector, "u01", 2, tiles[(2, "int")], 0)
        u23_2 = pair_add(nc.vector, "u23", 2, tiles[(2, "int")], 2)
        u01_3 = pair_add(nc.vector, "u01", 3, tiles[(3, "c01")], 0)
        u23_3 = pair_add(nc.vector, "u23", 3, tiles[(3, "c23")], 0)

        # --- matmuls ------------------------------------------------------
        n_mm = {0: 0, 1: 0}
        for bi, (s, lo, hi) in enumerate(BLOCKS):
            nch = hi - lo
            n_mm[s] += 4 * nch + (nch if bi < 2 else 2 * nch)
        mm_seen = {0: 0, 1: 0}

        def mm(s, ch, rhs):
            mm_seen[s] += 1
            nc.tensor.matmul(
                psum_feat[s],
                lhsT=w_sb[:, (s * NCHUNK + ch) * NF:(s * NCHUNK + ch + 1) * NF],
                rhs=rhs,
                start=(mm_seen[s] == 1),
                stop=(mm_seen[s] == n_mm[s]),
            )

        def fp8_group(bi, nm, njs):
            s, lo, hi = BLOCKS[bi]
            gw = GWS[bi]
            f8 = tiles[(bi, nm)]
            for j in range(njs):
                for chin in range(hi - lo):
                    mm(s, lo + chin, f8[:, j * gw + chin * P:j * gw + (chin + 1) * P])

        def red_group(bi, src):
            s, lo, hi = BLOCKS[bi]
            for chin in range(hi - lo):
                mm(s, lo + chin, src[:, chin * P:(chin + 1) * P])

        # PE queue in expected-readiness order
        fp8_group(0, "f8a", 2)
        fp8_group(0, "f8b", 2)
        red_group(0, v0_0)
        fp8_group(1, "f8a", 2)
        fp8_group(1, "f8b", 2)
        red_group(1, v0_1)
        fp8_group(2, "f8", 4)
        red_group(2, u01_2)
        red_group(2, u23_2)                # s0 stop
        red_group(3, u01_3)
        red_group(3, u23_3)
        fp8_group(3, "f8", 4)              # s1 stop (last piece's own work)

        assert mm_seen[0] == n_mm[0] and mm_seen[1] == n_mm[1]

        # --- epilogue: Prelus back-to-back on ACT, outs on sync -----------
        for s in range(NSW):
            nc.scalar.activation(
                out_sb[s * NF:(s + 1) * NF, :],
                psum_feat[s],
                mybir.ActivationFunctionType.Prelu,
                bias=bias_sb[s * NF:(s + 1) * NF, :],
                alpha=alpha_sb[s * NF:(s + 1) * NF, :],
            )
            nc.sync.dma_start(
                out=out[s * NF:(s + 1) * NF, :],
                in_=out_sb[s * NF:(s + 1) * NF, :],
            )


def _build():
    global _cached
    if _cached is not None:
        return _cached
    nc = bacc.Bacc(
        "TRN2",
        target_bir_lowering=False,
        debug=False,
        enable_asserts=False,
        num_devices=NCORES,
    )
    xi_ap = nc.dram_tensor("xi", [P, INT_COLS], I8, kind="ExternalInput").ap()
    xf_ap = nc.dram_tensor("xf", [P, FP8_COLS], FP8, kind="ExternalInput").ap()
    w_ap = nc.dram_tensor("w", [P, NSW * NCHUNK * NF], F16, kind="ExternalInput").ap()
    b_ap = nc.dram_tensor("bias", [OUT_F, 1], F32, kind="ExternalInput").ap()
    out_ap = nc.dram_tensor("out", [OUT_F, BS], F32, kind="ExternalOutput").ap()
    with tile.TileContext(nc, trace_sim=False) as tc:
        _kernel_body(tc, xi_ap, xf_ap, w_ap, b_ap, out_ap)
    nc.compile()
    _cached = nc
    return nc


def kernel(x, W, b):
    global last_results
    assert x.shape == (B, 1, NSW * NBINS * NDCT, HW, HW), x.shape
    nc = _build()

    # Host-side folding of the DCT matrices into the conv weights (tiny).
    Ct = _dct2(NDCT)                       # [f, t]
    Cs = _dct2(HW)                         # [p, i]
    Weff = np.einsum(
        "ft,pi,qj,sofpq->sotij", Ct, Cs, Cs, W.astype(np.float64), optimize=True
    ) / float(NBINS)
    Weff_k = Weff.reshape(NSW, NF, K) * QSCALE   # fold dequant scale
    w_dev = np.ascontiguousarray(
        Weff_k.reshape(NSW, NF, NCHUNK, P).transpose(3, 0, 2, 1).reshape(P, NSW * NCHUNK * NF)
    ).astype(NP_F16)
    bias_dev = np.ascontiguousarray(b.reshape(OUT_F, 1)).astype(np.float32)

    # Error-feedback quantization along c (fp8 slices first so their larger
    # residuals are absorbed by the later int8 slices).
    xs = x.reshape(B, NSW, NBINS, K).astype(np.float32) / np.float32(QSCALE)
    qi = np.zeros((B, NSW, 4, K), dtype=np.int8)
    qf = np.zeros((B, NSW, 4, K), dtype=NP_FP8)
    e = np.zeros((B, NSW, K), dtype=np.float32)
    for j, c in enumerate((4, 5, 6, 7)):
        v = xs[:, :, c] + e
        qc = v.astype(NP_FP8)
        e = v - qc.astype(np.float32)
        qf[:, :, j] = qc
    for c in range(4):
        v = xs[:, :, c] + e
        qc = np.clip(np.round(v), -127, 127)
        e = v - qc
        qi[:, :, c] = qc.astype(np.int8)

    qi = qi.reshape(B, NSW, 4, NCHUNK, P)
    qf = qf.reshape(B, NSW, 4, NCHUNK, P)

    in_maps = []
    for i in range(NCORES):
        sl = slice(i * BS, (i + 1) * BS)
        icols, fcols = [], []
        for kind, bi, nm, cs in PIECES:
            s, lo, hi = BLOCKS[bi]
            src = qi if kind == "i" else qf
            t = src[sl, s][:, list(cs), lo:hi, :]        # [b, nc, ch, kin]
            t = t.transpose(3, 1, 2, 0).reshape(P, len(cs) * (hi - lo) * BS)
            (icols if kind == "i" else fcols).append(t)
        xi_dev = np.ascontiguousarray(np.concatenate(icols, axis=1))
        xf_dev = np.ascontiguousarray(np.concatenate(fcols, axis=1))
        in_maps.append({"xi": xi_dev, "xf": xf_dev, "w": w_dev, "bias": bias_dev})
    res = run_bass_kernel_spmd(nc, in_maps, core_ids=list(range(NCORES)))
    last_results = res
    # device emits [s*64+o, b] per core; transpose back to [b, s*64+o]
    return np.concatenate([r["out"].T for r in res.results], axis=0)
